# revision 1
# baseline (speedup 1.0000x reference)
"""ALiBi attention (B=2, N=2048, C=1024, H=16, D=64) on 8 TRN2 NeuronCores.

Sharding: core i owns heads (2i, 2i+1) for both batches (4 [N,N] score blocks
per core). Q/K/V/first-proj are column-split over heads; output projection is
computed n-sharded after head-split AllToAlls of the per-head attention
outputs (the head-0 AllToAll + half the output projection overlap the head-1
attention compute).

Precision: the reference DIVIDES by scale (multiplies scores by sqrt(D)=8), so
score noise from bf16 rounding of Q/K would be ~0.2 absolute. All matmuls that
feed scores therefore use an exact bf16 hi/lo split: main = hi*hi (exact in the
PE's fp32 accumulator) plus one stacked cross-term matmul (hi*lo + lo*hi).

Layouts (transposed activations, contraction on partitions):
  xT [C, B*N] -> qT/kT [e, n] per head; v natural [m, e].
  pass1 (row-max for softmax stability): S1[n, m] via lhsT=Q rhs=K, DVE
    reduce_max -> -M[n], folded back into Q's aug row via PE transpose + DMA.
  pass2: S2[m, n] = qk - slope*n - M[n] + bf16(slope*m) via aug rows; ACT exp
    adds the fp32 residual of slope*m as per-partition bias; the AV matmul
    carries a ones-column in V so the softmax denominator falls out free.
  Q/K aug tiles are zero-padded to K=128: K<=66 matmuls keep the PE HAM
  throttled at 1.2 GHz, K=128 runs 2x faster for the same column count.
"""
import numpy as np
import ml_dtypes

import concourse.bacc as bacc
import concourse.mybir as mybir
import concourse.tile as tile
from concourse.bass_utils import run_bass_kernel_spmd

F32 = mybir.dt.float32
BF16 = mybir.dt.bfloat16
BF = ml_dtypes.bfloat16

B, N, C, H, D = 2, 2048, 1024, 16, 64
NCORES = 8
HL = H // NCORES          # heads per core (2)
BN = B * N                # 4096
NSH = BN // NCORES        # 512 output columns per core
CCH = C // 128            # 8 contraction chunks
NBH = B * HL              # 4 (batch, local-head) blocks per core
MC = N // 128             # 16 m-chunks per sequence
AX = mybir.AxisListType
ALU = mybir.AluOpType
ACT = mybir.ActivationFunctionType

_compiled = None


def _build():
    nc = bacc.Bacc("TRN2", target_bir_lowering=False, debug=False,
                   num_devices=NCORES)

    x_hi = nc.dram_tensor("x_hi", [128, CCH, BN], BF16, kind="ExternalInput")
    x_lo = nc.dram_tensor("x_lo", [128, CCH, BN], BF16, kind="ExternalInput")
    wq_hi = nc.dram_tensor("wq_hi", [128, CCH, 128], BF16, kind="ExternalInput")
    wq_lo = nc.dram_tensor("wq_lo", [128, CCH, 128], BF16, kind="ExternalInput")
    wk_hi = nc.dram_tensor("wk_hi", [128, CCH, 128], BF16, kind="ExternalInput")
    wk_lo = nc.dram_tensor("wk_lo", [128, CCH, 128], BF16, kind="ExternalInput")
    wv = nc.dram_tensor("wv", [128, CCH, 128], BF16, kind="ExternalInput")
    wp = nc.dram_tensor("wp", [128, CCH, C], BF16, kind="ExternalInput")
    bp_t = nc.dram_tensor("bp_t", [128, CCH], F32, kind="ExternalInput")
    qaug = nc.dram_tensor("qaug", [HL, 3, N], BF16, kind="ExternalInput")
    kaug = nc.dram_tensor("kaug", [HL, 3, N], BF16, kind="ExternalInput")
    mbias = nc.dram_tensor("mbias", [128, HL * MC], F32, kind="ExternalInput")
    ident = nc.dram_tensor("ident", [128, 128], F32, kind="ExternalInput")
    out_t = nc.dram_tensor("out", [C, NSH], F32, kind="ExternalOutput")

    with tile.TileContext(nc) as tc:
        with tc.tile_pool(name="wpool", bufs=1) as wpool, \
             tc.tile_pool(name="xpool", bufs=1) as xpool, \
             tc.tile_pool(name="qkpool", bufs=1) as qkpool, \
             tc.tile_pool(name="aux", bufs=2) as aux, \
             tc.tile_pool(name="attp", bufs=1) as attp, \
             tc.tile_pool(name="psum", bufs=1, space="PSUM") as psum, \
             tc.tile_pool(name="dram", bufs=1, space="DRAM") as dram:

            # ---------- resident weights / aux ----------
            wq_hi_sb = wpool.tile([128, CCH, 128], BF16)
            wq_lo_sb = wpool.tile([128, CCH, 128], BF16)
            wk_hi_sb = wpool.tile([128, CCH, 128], BF16)
            wk_lo_sb = wpool.tile([128, CCH, 128], BF16)
            wv_sb = wpool.tile([128, CCH, 128], BF16)
            wp_sb = wpool.tile([128, CCH, C], BF16)
            bp_sb = wpool.tile([128, CCH], F32)
            mbias_sb = wpool.tile([128, HL * MC], F32)
            ident_sb = wpool.tile([128, 128], F32)
            for sb_t, dr_t in ((wq_hi_sb, wq_hi), (wq_lo_sb, wq_lo),
                               (wk_hi_sb, wk_hi), (wk_lo_sb, wk_lo),
                               (wv_sb, wv)):
                nc.sync.dma_start(sb_t[:], dr_t[:, :])
            # not needed until attention / output projection: off the hot path
            for sb_t, dr_t in ((ident_sb, ident), (mbias_sb, mbias),
                               (bp_sb, bp_t), (wp_sb, wp)):
                nc.gpsimd.dma_start(sb_t[:], dr_t[:, :])

            # ---------- per-(batch, local-head) persistent tiles ----------
            QT, KT, QC, KC, VA, MP = [], [], [], [], [], []
            for i in range(NBH):
                j = i % HL
                # rows 0-63 hi part; 64-66 aug rows; 67-127 zero (K=128 pad)
                q = qkpool.tile([128, N], BF16, name=f"Qt{i}", tag=f"Qt{i}")
                k = qkpool.tile([128, N], BF16, name=f"Kt{i}", tag=f"Kt{i}")
                qc = qkpool.tile([128, N], BF16, name=f"Qc{i}", tag=f"Qc{i}")
                kc = qkpool.tile([128, N], BF16, name=f"Kc{i}", tag=f"Kc{i}")
                va = qkpool.tile([128, MC, 65], BF16, name=f"Va{i}", tag=f"Va{i}")
                mp = qkpool.tile([128, 32], F32, name=f"Mp{i}", tag=f"Mp{i}")
                nc.any.memset(q[64:128, :], 0.0)
                nc.any.memset(k[64:128, :], 0.0)
                # q rows 64-66: [-slope*n; -M placeholder (0); ones]
                nc.sync.dma_start(q[64:67, :], qaug[j, :, :])
                # k rows 64-66: [ones; ones; bf16(slope*m)]
                nc.sync.dma_start(k[64:67, :], kaug[j, :, :])
                nc.any.memset(va[:, :, 64:65], 1.0)
                QT.append(q); KT.append(k); QC.append(qc); KC.append(kc)
                VA.append(va); MP.append(mp)

            # ---------- projections (4 block-pairs of 1024 over B*N) ----------
            def proj_pair(bp_i):
                b = bp_i // 2
                nw = bp_i % 2         # 1024-block within batch
                col0 = bp_i * 1024
                xh, xl = [], []
                for c in range(CCH):
                    th = xpool.tile([128, 1024], BF16, name=f"xh{bp_i}_{c}",
                                    tag="xh", bufs=12)
                    tl = xpool.tile([128, 1024], BF16, name=f"xl{bp_i}_{c}",
                                    tag="xl", bufs=12)
                    (nc.sync if c % 2 == 0 else nc.scalar).dma_start(
                        th[:], x_hi[:, c, col0:col0 + 1024])
                    (nc.scalar if c % 2 == 0 else nc.sync).dma_start(
                        tl[:], x_lo[:, c, col0:col0 + 1024])
                    xh.append(th); xl.append(tl)

                cols = slice(nw * 1024, nw * 1024 + 1024)
                for w_hi_t, w_lo_t, T, TC, is_q in (
                        (wq_hi_sb, wq_lo_sb, QT, QC, True),
                        (wk_hi_sb, wk_lo_sb, KT, KC, False)):
                    ps = psum.tile([128, 1024], F32, name=f"pj{bp_i}_{int(is_q)}",
                                   tag="score", bufs=2)
                    for half in range(2):
                        hs = slice(half * 512, half * 512 + 512)
                        nmm = 3 * CCH
                        idx = 0
                        for c in range(CCH):
                            nc.tensor.matmul(ps[:, hs], w_hi_t[:, c, :],
                                             xh[c][:, hs],
                                             start=(idx == 0), stop=(idx == nmm - 1))
                            idx += 1
                        for c in range(CCH):
                            nc.tensor.matmul(ps[:, hs], w_lo_t[:, c, :],
                                             xh[c][:, hs],
                                             start=False, stop=(idx == nmm - 1))
                            idx += 1
                        for c in range(CCH):
                            nc.tensor.matmul(ps[:, hs], w_hi_t[:, c, :],
                                             xl[c][:, hs],
                                             start=False, stop=(idx == nmm - 1))
                            idx += 1
                        yield
                    for j in range(HL):
                        i = b * HL + j
                        rows = slice(64 * j, 64 * j + 64)
                        nc.any.tensor_copy(T[i][0:64, cols], ps[rows, :])
                        if is_q:   # Qc = [q_hi; q_lo]
                            nc.any.tensor_copy(TC[i][0:64, cols], T[i][0:64, cols])
                            nc.vector.tensor_sub(TC[i][64:128, cols], ps[rows, :],
                                                 T[i][0:64, cols])
                        else:      # Kc = [k_lo; k_hi]
                            nc.any.tensor_copy(TC[i][64:128, cols], T[i][0:64, cols])
                            nc.vector.tensor_sub(TC[i][0:64, cols], ps[rows, :],
                                                 T[i][0:64, cols])
                    yield

                # v in natural [m, e] layout
                for mt in range(8):
                    vps = psum.tile([128, 128], F32, name=f"v{bp_i}_{mt}",
                                    tag="avsm", bufs=2)
                    for c in range(CCH):
                        nc.tensor.matmul(vps[:], xh[c][:, mt * 128:(mt + 1) * 128],
                                         wv_sb[:, c, :],
                                         start=(c == 0), stop=(c == CCH - 1))
                    mc = nw * 8 + mt
                    for j in range(HL):
                        i = b * HL + j
                        nc.any.tensor_copy(VA[i][:, mc, 0:64],
                                           vps[:, 64 * j:64 * j + 64])
                    if mt % 2 == 1:
                        yield

            # ---------- attention ----------
            # head-split AllToAll buffers: ag[j] carries local head j's rows
            ag_in = [dram.tile([NCORES, 64, NSH], BF16, name=f"agi{j}")
                     for j in range(HL)]
            ag_out = [dram.tile([NCORES, 64, NSH], BF16, name=f"ago{j}")
                      for j in range(HL)]

            def pass1(i):
                Q, K, Mpt = QT[i], KT[i], MP[i]
                for nt in range(16):
                    for half in range(2):
                        ps = psum.tile([128, 1024], F32, tag="score", bufs=2,
                                       name=f"p1_{i}_{nt}_{half}")
                        for mb in range(2):
                            m0 = (half * 2 + mb) * 512
                            nc.tensor.matmul(ps[:, mb * 512:(mb + 1) * 512],
                                             Q[:, nt * 128:(nt + 1) * 128],
                                             K[:, m0:m0 + 512],
                                             start=True, stop=True)
                        nc.vector.tensor_reduce(
                            Mpt[:, nt * 2 + half:nt * 2 + half + 1], ps[:, :],
                            axis=AX.X, op=ALU.max)
                        yield
                mneg = aux.tile([128, 16], F32, tag="mneg", name=f"mneg{i}")
                nc.vector.tensor_reduce(
                    mneg[:], Mpt[:].rearrange("p (a b) -> p a b", b=2),
                    axis=AX.X, op=ALU.max, negate=True)
                trp = psum.tile([16, 128], F32, tag="avsm", bufs=2, name=f"trp{i}")
                nc.tensor.transpose(trp[:], mneg[:], ident_sb[:])
                mrow16 = aux.tile([16, 128], BF16, tag="mrow16", name=f"mr{i}")
                nc.any.tensor_copy(mrow16[:], trp[:])
                nc.gpsimd.dma_start(QT[i][65:66, :], mrow16[:, :])
                yield

            def pass2(i):
                b, j = divmod(i, HL)
                Q, K, Qc, Kc, Va = QT[i], KT[i], QC[i], KC[i], VA[i]
                for nb in range(2):
                    n0 = nb * 1024
                    avp = psum.tile([65, 1024], F32, tag="avsm", bufs=2,
                                    name=f"av_{i}_{nb}")
                    at_q = []

                    def emit_av(mc, at):
                        for hf in range(2):
                            hs = slice(hf * 512, hf * 512 + 512)
                            nc.tensor.matmul(avp[:, hs], Va[:, mc, :], at[:, hs],
                                             start=(mc == 0), stop=(mc == MC - 1))

                    for mc in range(MC):
                        s2 = psum.tile([128, 1024], F32, tag="score", bufs=2,
                                       name=f"s2_{i}_{nb}_{mc}")
                        for hf in range(2):
                            hs = slice(hf * 512, hf * 512 + 512)
                            ns = slice(n0 + hf * 512, n0 + hf * 512 + 512)
                            nc.tensor.matmul(s2[:, hs],
                                             K[:, mc * 128:(mc + 1) * 128],
                                             Q[:, ns], start=True, stop=False)
                            nc.tensor.matmul(s2[:, hs],
                                             Kc[:, mc * 128:(mc + 1) * 128],
                                             Qc[:, ns], start=False, stop=True)
                        at = attp.tile([128, 1024], BF16, tag="att", bufs=3,
                                       name=f"at_{i}_{nb}_{mc}")
                        nc.scalar.activation(at[:], s2[:], ACT.Exp,
                                             bias=mbias_sb[:, j * MC + mc:j * MC + mc + 1],
                                             scale=1.0)
                        # av for the PREVIOUS chunk: its exp has had a full
                        # s2-round to drain, so the PE never waits on ACT
                        at_q.append((mc, at))
                        if len(at_q) > 1:
                            emit_av(*at_q.pop(0))
                        yield
                    emit_av(*at_q.pop(0))
                    # normalize: reciprocal spread over 32 partitions (DVE
                    # reciprocal is ~8 cyc/elem/lane)
                    lrow = aux.tile([1, 1024], F32, tag="lrow", bufs=2, name=f"lr_{i}_{nb}")
                    nc.any.tensor_copy(lrow[0:1, :], avp[64:65, :])
                    l32 = aux.tile([32, 32], F32, tag="l32", bufs=2, name=f"l32_{i}_{nb}")
                    nc.gpsimd.dma_start(l32[:, :], lrow[0:1, :])
                    r32 = aux.tile([32, 32], F32, tag="r32", bufs=2, name=f"r32_{i}_{nb}")
                    nc.vector.reciprocal(r32[:], l32[:])
                    linv = aux.tile([1, 1024], F32, tag="linv", bufs=2, name=f"li_{i}_{nb}")
                    nc.gpsimd.dma_start(linv[0:1, :], r32[:, :])
                    lb = aux.tile([64, 1024], F32, tag="lb", bufs=2, name=f"lb_{i}_{nb}")
                    nc.gpsimd.partition_broadcast(lb[:], linv[0:1, :])
                    gt = aux.tile([64, 1024], BF16, tag="gt", bufs=2, name=f"gt_{i}_{nb}")
                    nc.vector.tensor_mul(gt[:], avp[0:64, :], lb[:])
                    for hf in range(2):
                        s = b * 4 + nb * 2 + hf
                        nc.sync.dma_start(ag_in[j][s, :, :],
                                          gt[:, hf * 512:hf * 512 + 512])
                    yield

            gt_in = attp.tile([128, CCH, NSH], BF16, tag="gtin", bufs=1)

            def emit_a2a(j):
                nc.gpsimd.collective_compute(
                    "AllToAll", ALU.bypass,
                    replica_groups=[list(range(NCORES))],
                    ins=[ag_in[j].opt()],
                    outs=[ag_out[j].opt()],
                )
                rows = slice(64 * j, 64 * j + 64)
                # head-0's redistribution runs mid-kernel: keep it OFF the
                # gpsimd queue (pass2's normalize-chain DMAs live there)
                engs = [nc.sync] if j == 0 else                     [nc.sync, nc.gpsimd, nc.scalar]
                for c in range(CCH):
                    engs[c % len(engs)].dma_start(gt_in[rows, c, :],
                                                  ag_out[j][c, :, :])

            def drive(*gens_weights):
                """Round-robin generators with weights until all exhausted."""
                gens = [[g, w] for g, w in gens_weights]
                while gens:
                    for gw in list(gens):
                        g, w = gw
                        for _ in range(w):
                            try:
                                next(g)
                            except StopIteration:
                                gens.remove(gw)
                                break

            def chain(*gens):
                for g in gens:
                    yield from g

            # batch-0 proj; then batch-1 proj interleaved with head-0 pass1;
            # then pass2(i) interleaved with the next pass1
            drive((chain(proj_pair(0), proj_pair(1)), 1))
            drive((chain(proj_pair(2), proj_pair(3)), 1),
                  (pass1(0), 3))
            drive((pass2(0), 1), (chain(pass1(2), pass1(1)), 2))
            drive((pass2(2), 1), (pass1(3), 1))
            emit_a2a(0)
            drive((pass2(1), 1))
            drive((pass2(3), 1))
            emit_a2a(1)

            # ---------- output projection ----------
            for et in range(CCH):
                yps = psum.tile([128, 512], F32, tag="avsm", bufs=2, name=f"y{et}")
                for c in range(CCH):
                    nc.tensor.matmul(yps[:], wp_sb[:, c, et * 128:(et + 1) * 128],
                                     gt_in[:, c, :],
                                     start=(c == 0), stop=(c == CCH - 1))
                ysb = aux.tile([128, 512], F32, tag="y", name=f"ysb{et}")
                nc.scalar.activation(ysb[:], yps[:], ACT.Identity,
                                     bias=bp_sb[:, et:et + 1], scale=1.0)
                nc.sync.dma_start(out_t[et * 128:(et + 1) * 128, :], ysb[:])

    nc.compile()
    return nc


def _get_nc():
    global _compiled
    if _compiled is None:
        _compiled = _build()
    return _compiled


def _alibi_slopes():
    x = (2 ** 8) ** (1.0 / H)
    return np.array([1.0 / x ** (i + 1) for i in range(H)], dtype=np.float64)


def _chunked(a):
    """[C, F] -> [128, CCH, F] (partition, c-chunk, free)."""
    Cdim, F = a.shape
    return np.ascontiguousarray(a.reshape(CCH, 128, F).transpose(1, 0, 2))


def _split(a):
    hi = a.astype(BF)
    lo = (a - hi.astype(np.float32)).astype(BF)
    return hi, lo


def _make_in_maps(x, Wq, Wk, Wv, Wp, bp):
    x = np.asarray(x, dtype=np.float32)
    xT = np.ascontiguousarray(x.reshape(BN, C).T)          # [C, BN]
    xch = _chunked(xT)
    xch_hi, xch_lo = _split(xch)

    slopes = _alibi_slopes()
    n_arr = np.arange(N, dtype=np.float64)
    p_arr = np.arange(128, dtype=np.float64)

    wp_ch = _chunked(np.ascontiguousarray(np.asarray(Wp, np.float32).T)).astype(BF)
    bp_tile = np.ascontiguousarray(
        np.asarray(bp, np.float32).reshape(CCH, 128).T)
    identity = np.eye(128, dtype=np.float32)

    in_maps = []
    for core in range(NCORES):
        e0 = core * 128
        wqT = np.ascontiguousarray((8.0 * np.asarray(Wq, np.float32)[e0:e0 + 128]).T)
        wkT = np.ascontiguousarray(np.asarray(Wk, np.float32)[e0:e0 + 128].T)
        wvT = np.ascontiguousarray(np.asarray(Wv, np.float32)[e0:e0 + 128].T)
        wq_h, wq_l = _split(_chunked(wqT))
        wk_h, wk_l = _split(_chunked(wkT))

        s = slopes[core * HL: core * HL + HL]               # [HL]
        qa = np.zeros((HL, 3, N), dtype=BF)
        ka = np.zeros((HL, 3, N), dtype=BF)
        # pass2's K=128 main matmul already adds bf16(slope*m) via k row 66;
        # the exp bias supplies only the fp32 residual so the total is exact
        mb = np.zeros((128, HL * MC), dtype=np.float32)
        for j in range(HL):
            qa[j, 0] = (-s[j] * n_arr).astype(BF)   # -slope*n
            qa[j, 1] = 0.0                          # -M placeholder
            qa[j, 2] = 1.0
            ka[j, 0] = 1.0
            ka[j, 1] = 1.0
            ka[j, 2] = (s[j] * n_arr).astype(BF)    # bf16(slope*m)
            for c in range(MC):
                exact = (s[j] * (128 * c + p_arr)).astype(np.float32)
                mb[:, j * MC + c] = exact - exact.astype(BF).astype(np.float32)

        in_maps.append({
            "x_hi": xch_hi, "x_lo": xch_lo,
            "wq_hi": wq_h, "wq_lo": wq_l,
            "wk_hi": wk_h, "wk_lo": wk_l,
            "wv": _chunked(wvT).astype(BF),
            "wp": wp_ch, "bp_t": bp_tile,
            "qaug": qa, "kaug": ka, "mbias": mb,
            "ident": identity,
        })
    return in_maps


def run(x, Wq, Wk, Wv, Wp, bp, trace=False, tmpdir=None):
    nc = _get_nc()
    in_maps = _make_in_maps(x, Wq, Wk, Wv, Wp, bp)
    kwargs = {}
    if trace:
        kwargs = {"trace": True, "tmpdir": tmpdir}
    res = run_bass_kernel_spmd(nc, in_maps, core_ids=list(range(NCORES)), **kwargs)
    yT = np.concatenate([res.results[i]["out"] for i in range(NCORES)], axis=1)
    out = np.ascontiguousarray(yT.T).reshape(B, N, C).astype(np.float32)
    return out, res


def kernel(x, Wq, Wk, Wv, Wp, bp):
    out, _ = run(x, Wq, Wk, Wv, Wp, bp)
    return out



# revision 7
# speedup vs baseline: 1.0828x; 1.0828x over previous
"""ALiBi attention (B=2, N=2048, C=1024, H=16, D=64) on 8 TRN2 NeuronCores.

Sharding: core i owns heads (2i, 2i+1) for both batches (4 [N,N] score blocks
per core). Q/K/V are column-split over heads; the output projection is
n-sharded after head-split AllToAlls of the per-head attention outputs.

Precision: all score-path matmuls (projections, both score passes) run in
fp32r (TF32-like, ~13 mantissa bits, 1 cyc/row at 512-col moving dim), which
is accurate enough for the x8-scaled scores (abs score err ~0.01) without any
hi/lo split machinery. Attention probs and V use fp16; a2a payload and output
projection use bf16.

Token mapping is interleaved so collectives split per score-block: core i's
output columns are batch-0 tokens [256i,256i+256) then batch-1 tokens
[256i,256i+256). Each of the 4 per-core attention blocks (b,j) fires its own
small AllToAll right when it finishes; the first half of the output projection
(batch-0 columns) overlaps the last attention block, so only a ~256-column
projection remains after the final collective.

Softmax stability: pass1 computes S1[n,m] = qk - slope*n + fp16(slope*m) via
aug rows and DVE row-max -> -M folded back into Q's aug row (PE transpose).
pass2 computes S2[m,n] = qk - slope*n - M + fp16(slope*m); ACT exp adds the
fp32 residual of slope*m as per-partition bias so the total bias is exact.
A ones-column in V makes the softmax denominator fall out of the AV matmul.
"""
import numpy as np
import ml_dtypes

import concourse.bacc as bacc
import concourse.mybir as mybir
import concourse.tile as tile
from concourse.bass_utils import run_bass_kernel_spmd

F32 = mybir.dt.float32
F32R = mybir.dt.float32r
BF16 = mybir.dt.bfloat16
FP16 = mybir.dt.float16
BF = ml_dtypes.bfloat16

B, N, C, H, D = 2, 2048, 1024, 16, 64
NCORES = 8
HL = H // NCORES          # heads per core (2)
BN = B * N                # 4096
NSH = BN // NCORES        # 512 output columns per core
TSH = NSH // B            # 256 tokens per (core, batch)
CCH = C // 128            # 8 contraction chunks
NBH = B * HL              # 4 (batch, local-head) blocks per core
MC = N // 128             # 16 m-chunks per sequence
AX = mybir.AxisListType
ALU = mybir.AluOpType
ACT = mybir.ActivationFunctionType

_compiled = None


def _build():
    nc = bacc.Bacc("TRN2", target_bir_lowering=False, debug=False,
                   num_devices=NCORES)

    x_in = nc.dram_tensor("x_in", [128, CCH, BN], F32R, kind="ExternalInput")
    wq = nc.dram_tensor("wq", [128, CCH, 128], F32R, kind="ExternalInput")
    wk = nc.dram_tensor("wk", [128, CCH, 128], F32R, kind="ExternalInput")
    wv = nc.dram_tensor("wv", [128, CCH, 128], F32R, kind="ExternalInput")
    wp = nc.dram_tensor("wp", [128, CCH, C], BF16, kind="ExternalInput")
    bp_t = nc.dram_tensor("bp_t", [128, CCH], F32, kind="ExternalInput")
    qaug = nc.dram_tensor("qaug", [HL, 3, N], F32R, kind="ExternalInput")
    kaug = nc.dram_tensor("kaug", [HL, 3, N], F32R, kind="ExternalInput")
    mbias = nc.dram_tensor("mbias", [128, HL * MC], F32, kind="ExternalInput")
    zrows = nc.dram_tensor("zrows", [64, N], F32R, kind="ExternalInput")
    ident = nc.dram_tensor("ident", [128, 128], F32, kind="ExternalInput")
    out_t = nc.dram_tensor("out", [C, NSH], F32, kind="ExternalOutput")

    with tile.TileContext(nc) as tc:
        with tc.tile_pool(name="wpool", bufs=1) as wpool, \
             tc.tile_pool(name="xpool", bufs=1) as xpool, \
             tc.tile_pool(name="qkpool", bufs=1) as qkpool, \
             tc.tile_pool(name="aux", bufs=2) as aux, \
             tc.tile_pool(name="attp", bufs=1) as attp, \
             tc.tile_pool(name="psum", bufs=1, space="PSUM") as psum, \
             tc.tile_pool(name="dram", bufs=1, space="DRAM") as dram:

            # ---------- resident weights / aux ----------
            wq_sb = wpool.tile([128, CCH, 128], F32R)
            wk_sb = wpool.tile([128, CCH, 128], F32R)
            wv_sb = wpool.tile([128, CCH, 128], F32R)
            wp_sb = wpool.tile([128, CCH, C], BF16)
            bp_sb = wpool.tile([128, CCH], F32)
            mbias_sb = wpool.tile([128, HL * MC], F32)
            ident_sb = wpool.tile([128, 128], F32)
            for sb_t, dr_t in ((wq_sb, wq), (wk_sb, wk), (wv_sb, wv)):
                nc.sync.dma_start(sb_t[:], dr_t[:, :])
            # not needed until attention / output projection: off the hot path
            for sb_t, dr_t in ((ident_sb, ident), (mbias_sb, mbias),
                               (bp_sb, bp_t), (wp_sb, wp)):
                nc.gpsimd.dma_start(sb_t[:], dr_t[:, :])

            # ---------- per-(batch, local-head) persistent tiles ----------
            QT, KT, VA, MP = [], [], [], []
            for i in range(NBH):
                j = i % HL
                # rows 0-63 head dims; 64-66 aug rows; 67-127 zero (K=128 pad)
                q = qkpool.tile([128, N], F32R, name=f"Qt{i}", tag=f"Qt{i}")
                k = qkpool.tile([128, N], F32R, name=f"Kt{i}", tag=f"Kt{i}")
                va = qkpool.tile([128, MC, 65], FP16, name=f"Va{i}", tag=f"Va{i}")
                mp = qkpool.tile([128, 64], F32, name=f"Mp{i}", tag=f"Mp{i}")
                nc.gpsimd.dma_start(q[64:128, :], zrows[:, :])
                nc.gpsimd.dma_start(k[64:128, :], zrows[:, :])
                # q rows 64-66: [-slope*n; -M placeholder (0); ones]
                nc.sync.dma_start(q[64:67, :], qaug[j, :, :])
                # k rows 64-66: [ones; ones; f32(fp16(slope*m))]
                nc.sync.dma_start(k[64:67, :], kaug[j, :, :])
                nc.any.memset(va[:, :, 64:65], 1.0)
                QT.append(q); KT.append(k); VA.append(va); MP.append(mp)

            # ---------- projections (4 pairs of 1024 tokens over B*N) ----------
            def proj_pair(bp_i):
                b = bp_i // 2
                nw = bp_i % 2         # 1024-block within batch
                col0 = bp_i * 1024
                xh = []
                for c in range(CCH):
                    th = xpool.tile([128, 1024], F32R, name=f"x{bp_i}_{c}",
                                    tag="xt", bufs=12)
                    (nc.sync if c % 2 == 0 else nc.scalar).dma_start(
                        th[:], x_in[:, c, col0:col0 + 1024])
                    xh.append(th)

                cols = slice(nw * 1024, nw * 1024 + 1024)
                for w_t, T, is_q in ((wq_sb, QT, True), (wk_sb, KT, False)):
                    for half in range(2):
                        hs = slice(half * 512, half * 512 + 512)
                        ps = psum.tile([128, 512], F32, tag="sc", bufs=4,
                                       name=f"pj{bp_i}_{int(is_q)}_{half}")
                        for c in range(CCH):
                            nc.tensor.matmul(ps[:], w_t[:, c, :],
                                             xh[c][:, hs],
                                             start=(c == 0), stop=(c == CCH - 1))
                        ccols = slice(nw * 1024 + half * 512,
                                      nw * 1024 + half * 512 + 512)
                        for j in range(HL):
                            i = b * HL + j
                            rows = slice(64 * j, 64 * j + 64)
                            nc.any.tensor_copy(T[i][0:64, ccols], ps[rows, :])
                        yield

                # v in natural [m, e] layout
                for mt in range(8):
                    vps = psum.tile([128, 128], F32, tag="sc", bufs=4,
                                    name=f"v{bp_i}_{mt}")
                    for c in range(CCH):
                        nc.tensor.matmul(vps[:],
                                         xh[c][:, mt * 128:(mt + 1) * 128],
                                         wv_sb[:, c, :],
                                         start=(c == 0), stop=(c == CCH - 1))
                    mc = nw * 8 + mt
                    for j in range(HL):
                        i = b * HL + j
                        nc.any.tensor_copy(VA[i][:, mc, 0:64],
                                           vps[:, 64 * j:64 * j + 64])
                    if mt % 2 == 1:
                        yield

            # ---------- attention ----------
            # per-block head-split AllToAll buffers: block i = (b, j) carries
            # head (2*core+j), batch b rows for all destination cores
            ag_in = [dram.tile([NCORES, 64, TSH], BF16, name=f"agi{i}")
                     for i in range(NBH)]
            ag_out = [dram.tile([NCORES, 64, TSH], BF16, name=f"ago{i}")
                      for i in range(NBH)]
            # gathered attention outputs for my tokens: [channel, token]
            # cols 0:256 batch-0 tokens, 256:512 batch-1 tokens
            gt_in = attp.tile([128, CCH, NSH], BF16, tag="gtin", bufs=1)

            def pass1(i):
                Q, K, Mpt = QT[i], KT[i], MP[i]
                for nt in range(16):
                    for mb in range(4):
                        ps = psum.tile([128, 512], F32, tag="sc", bufs=4,
                                       name=f"p1_{i}_{nt}_{mb}")
                        nc.tensor.matmul(ps[:],
                                         Q[:, nt * 128:(nt + 1) * 128],
                                         K[:, mb * 512:(mb + 1) * 512],
                                         start=True, stop=True)
                        nc.vector.tensor_reduce(
                            Mpt[:, nt * 4 + mb:nt * 4 + mb + 1], ps[:, :],
                            axis=AX.X, op=ALU.max)
                        if mb % 2 == 1:
                            yield
                mneg = aux.tile([128, 16], F32, tag="mneg", name=f"mneg{i}")
                nc.vector.tensor_reduce(
                    mneg[:], Mpt[:].rearrange("p (a b) -> p a b", b=4),
                    axis=AX.X, op=ALU.max, negate=True)
                trp = psum.tile([16, 128], F32, tag="sc", bufs=4, name=f"trp{i}")
                nc.tensor.transpose(trp[:], mneg[:], ident_sb[:])
                mrow = aux.tile([16, 128], F32R, tag="mrow", name=f"mr{i}")
                nc.any.tensor_copy(mrow[:], trp[:])
                nc.gpsimd.dma_start(QT[i][65:66, :], mrow[:, :])
                yield

            def pass2(i):
                b, j = divmod(i, HL)
                Q, K, Va = QT[i], KT[i], VA[i]
                for nb in range(2):
                    n0 = nb * 1024
                    avp = psum.tile([65, 1024], F32, tag="av", bufs=2,
                                    name=f"av_{i}_{nb}")
                    at_q = []

                    def emit_av(mc, at, hf):
                        hs = slice(hf * 512, hf * 512 + 512)
                        nc.tensor.matmul(avp[:, hs], Va[:, mc, :], at[:],
                                         start=(mc == 0), stop=(mc == MC - 1))

                    for mc in range(MC):
                        for hf in range(2):
                            s2 = psum.tile([128, 512], F32, tag="sc", bufs=4,
                                           name=f"s2_{i}_{nb}_{mc}_{hf}")
                            ns = slice(n0 + hf * 512, n0 + hf * 512 + 512)
                            nc.tensor.matmul(s2[:],
                                             K[:, mc * 128:(mc + 1) * 128],
                                             Q[:, ns],
                                             start=True, stop=True)
                            at = attp.tile([128, 512], FP16, tag="att", bufs=6,
                                           name=f"at_{i}_{nb}_{mc}_{hf}")
                            nc.scalar.activation(
                                at[:], s2[:], ACT.Exp,
                                bias=mbias_sb[:, j * MC + mc:j * MC + mc + 1],
                                scale=1.0)
                            # av for an OLDER chunk: its exp has had time to
                            # drain, so the PE never waits on ACT
                            at_q.append((mc, at, hf))
                            if len(at_q) > 2:
                                emit_av(*at_q.pop(0))
                        yield
                    while at_q:
                        emit_av(*at_q.pop(0))
                    # normalize: reciprocal spread over 32 partitions
                    lrow = aux.tile([1, 1024], F32, tag="lrow", bufs=2,
                                    name=f"lr_{i}_{nb}")
                    nc.any.tensor_copy(lrow[0:1, :], avp[64:65, :])
                    l32 = aux.tile([32, 32], F32, tag="l32", bufs=2,
                                   name=f"l32_{i}_{nb}")
                    nc.sync.dma_start(l32[:, :], lrow[0:1, :])
                    r32 = aux.tile([32, 32], F32, tag="r32", bufs=2,
                                   name=f"r32_{i}_{nb}")
                    nc.vector.reciprocal(r32[:], l32[:])
                    linv = aux.tile([1, 1024], F32, tag="linv", bufs=2,
                                    name=f"li_{i}_{nb}")
                    nc.sync.dma_start(linv[0:1, :], r32[:, :])
                    lb = aux.tile([64, 1024], F32, tag="lb", bufs=2,
                                  name=f"lb_{i}_{nb}")
                    nc.gpsimd.partition_broadcast(lb[:], linv[0:1, :])
                    gt = aux.tile([64, 1024], BF16, tag="gt", bufs=2,
                                  name=f"gt_{i}_{nb}")
                    nc.vector.tensor_mul(gt[:], avp[0:64, :], lb[:])
                    for sf in range(4):
                        s = nb * 4 + sf
                        (nc.sync if sf % 2 == 0 else nc.scalar).dma_start(
                            ag_in[i][s, :, :], gt[:, sf * 256:sf * 256 + 256])
                    yield

            def emit_a2a(i):
                b, j = divmod(i, HL)
                nc.gpsimd.collective_compute(
                    "AllToAll", ALU.bypass,
                    replica_groups=[list(range(NCORES))],
                    ins=[ag_in[i].opt()],
                    outs=[ag_out[i].opt()],
                )
                rows = slice(64 * j, 64 * j + 64)
                csl = slice(TSH * b, TSH * b + TSH)
                for c in range(CCH):
                    (nc.sync if c % 2 == 0 else nc.scalar).dma_start(
                        gt_in[rows, c, csl], ag_out[i][c, :, :])

            def outproj(half):
                csl = slice(half * TSH, half * TSH + TSH)
                for et in range(CCH):
                    yps = psum.tile([128, TSH], F32, tag="sc", bufs=4,
                                    name=f"y{half}_{et}")
                    for c in range(CCH):
                        nc.tensor.matmul(yps[:],
                                         wp_sb[:, c, et * 128:(et + 1) * 128],
                                         gt_in[:, c, csl],
                                         start=(c == 0), stop=(c == CCH - 1))
                    ysb = aux.tile([128, TSH], F32, tag="y", name=f"ysb{half}_{et}")
                    nc.scalar.activation(ysb[:], yps[:], ACT.Identity,
                                         bias=bp_sb[:, et:et + 1], scale=1.0)
                    nc.sync.dma_start(out_t[et * 128:(et + 1) * 128, csl], ysb[:])
                    yield

            def drive(*gens_weights):
                """Round-robin generators with weights until all exhausted."""
                gens = [[g, w] for g, w in gens_weights]
                while gens:
                    for gw in list(gens):
                        g, w = gw
                        for _ in range(w):
                            try:
                                next(g)
                            except StopIteration:
                                gens.remove(gw)
                                break

            def chain(*gens):
                for g in gens:
                    yield from g

            # batch-0 proj; then batch-1 proj interleaved with batch-0 pass1;
            # then pass2 blocks in order, each firing its own AllToAll;
            # output projection halves overlap the later blocks
            drive((chain(proj_pair(0), proj_pair(1)), 1))
            drive((chain(proj_pair(2), proj_pair(3)), 1),
                  (chain(pass1(0), pass1(1)), 2))
            g22 = pass2(2)
            drive((pass2(0), 1), (chain(pass1(2), pass1(3)), 1))
            emit_a2a(0)
            drive((pass2(1), 2), (g22, 1))
            emit_a2a(1)
            drive((g22, 1))
            emit_a2a(2)
            drive((pass2(3), 2), (outproj(0), 1))
            emit_a2a(3)
            drive((outproj(1), 1))

    nc.compile()
    return nc


def _get_nc():
    global _compiled
    if _compiled is None:
        _compiled = _build()
    return _compiled


def _alibi_slopes():
    x = (2 ** 8) ** (1.0 / H)
    return np.array([1.0 / x ** (i + 1) for i in range(H)], dtype=np.float64)


def _chunked(a):
    """[C, F] -> [128, CCH, F] (partition, c-chunk, free)."""
    Cdim, F = a.shape
    return np.ascontiguousarray(a.reshape(CCH, 128, F).transpose(1, 0, 2))


def _make_in_maps(x, Wq, Wk, Wv, Wp, bp):
    x = np.asarray(x, dtype=np.float32)
    xT = np.ascontiguousarray(x.reshape(BN, C).T)          # [C, BN]
    xch = _chunked(xT)

    slopes = _alibi_slopes()
    n_arr = np.arange(N, dtype=np.float64)
    p_arr = np.arange(128, dtype=np.float64)

    wp_ch = _chunked(np.ascontiguousarray(np.asarray(Wp, np.float32).T)).astype(BF)
    bp_tile = np.ascontiguousarray(
        np.asarray(bp, np.float32).reshape(CCH, 128).T)
    identity = np.eye(128, dtype=np.float32)

    in_maps = []
    for core in range(NCORES):
        e0 = core * 128
        wqT = np.ascontiguousarray((8.0 * np.asarray(Wq, np.float32)[e0:e0 + 128]).T)
        wkT = np.ascontiguousarray(np.asarray(Wk, np.float32)[e0:e0 + 128].T)
        wvT = np.ascontiguousarray(np.asarray(Wv, np.float32)[e0:e0 + 128].T)

        s = slopes[core * HL: core * HL + HL]               # [HL]
        qa = np.zeros((HL, 3, N), dtype=np.float32)
        ka = np.zeros((HL, 3, N), dtype=np.float32)
        # pass2's matmul adds fp16(slope*m) via k row 66 (fp16 values are
        # exact in fp32r); the exp bias supplies the fp32 residual
        mb = np.zeros((128, HL * MC), dtype=np.float32)
        for j in range(HL):
            qa[j, 0] = (-s[j] * n_arr).astype(np.float32)   # -slope*n
            qa[j, 1] = 0.0                                  # -M placeholder
            qa[j, 2] = 1.0
            ka[j, 0] = 1.0
            ka[j, 1] = 1.0
            ka[j, 2] = (s[j] * n_arr).astype(np.float16).astype(np.float32)
            for c in range(MC):
                exact = (s[j] * (128 * c + p_arr))
                mb[:, j * MC + c] = (exact -
                                     exact.astype(np.float16).astype(np.float64)
                                     ).astype(np.float32)

        in_maps.append({
            "x_in": xch, "zrows": np.zeros((64, N), dtype=np.float32),
            "wq": _chunked(wqT), "wk": _chunked(wkT), "wv": _chunked(wvT),
            "wp": wp_ch, "bp_t": bp_tile,
            "qaug": qa, "kaug": ka, "mbias": mb,
            "ident": identity,
        })
    return in_maps


def run(x, Wq, Wk, Wv, Wp, bp, trace=False, tmpdir=None):
    nc = _get_nc()
    in_maps = _make_in_maps(x, Wq, Wk, Wv, Wp, bp)
    kwargs = {}
    if trace:
        kwargs = {"trace": True, "tmpdir": tmpdir}
    res = run_bass_kernel_spmd(nc, in_maps, core_ids=list(range(NCORES)), **kwargs)
    # core i columns: [0:256] = batch-0 tokens [256i,256i+256),
    #                 [256:512] = batch-1 tokens [256i,256i+256)
    out = np.empty((B, N, C), dtype=np.float32)
    for i in range(NCORES):
        yT = res.results[i]["out"]                          # [C, NSH]
        for b in range(B):
            out[b, TSH * i:TSH * i + TSH, :] = yT[:, b * TSH:(b + 1) * TSH].T
    return out, res


def kernel(x, Wq, Wk, Wv, Wp, bp):
    out, _ = run(x, Wq, Wk, Wv, Wp, bp)
    return out


# revision 9
# speedup vs baseline: 1.1725x; 1.0828x over previous
"""ALiBi attention (B=2, N=2048, C=1024, H=16, D=64) on 8 TRN2 NeuronCores.

Sharding: core i owns heads (2i, 2i+1) for both batches (4 [N,N] score blocks
per core). Q/K/V are column-split over heads; the output projection is
n-sharded after head-split AllToAlls of the per-head attention outputs.

Precision: score-path matmuls (Q/K/V projections, both score passes) run in
fp32r (TF32-like, ~13 mantissa bits, full rate at 512-col moving dim), which
is accurate enough for the x8-scaled scores without hi/lo split machinery.
Attention probs, V, a2a payload and the output projection use bf16.

Token mapping is interleaved so collectives split per score-block: core i's
output columns are batch-0 tokens [256i,256i+256) then batch-1 tokens
[256i,256i+256). Each of the 4 attention blocks (b,j) fires its own small
AllToAll when it finishes; outproj half 0 runs inside the last collective's
flight window, so only a ~256-column projection tails the final AllToAll.
A tiny warmup AllToAll at kernel start absorbs the collective cold-start.

Softmax stability: pass1 computes S1[n,m] = qk - slope*n + fp16(slope*m) via
aug rows (K=67 contraction, no zero padding) and DVE row-max -> -M folded
back into Q's aug row via PE transpose. pass2 computes S2[m,n] = qk - slope*n
- M + fp16(slope*m); ACT exp adds the fp32 residual of slope*m as
per-partition bias so the total bias is exact (fp16 values are exact in
fp32r). A ones-column in V makes the softmax denominator fall out of the AV
matmul. V is projected in [e,m] layout (512-wide moving dim; 128-wide fp32r
matmuls run at 1/4 rate) and PE-transposed into the [m,e] AV operand.
"""
import numpy as np
import ml_dtypes

import concourse.bacc as bacc
import concourse.mybir as mybir
import concourse.tile as tile
from concourse.bass_utils import run_bass_kernel_spmd

F32 = mybir.dt.float32
F32R = mybir.dt.float32r
BF16 = mybir.dt.bfloat16
BF = ml_dtypes.bfloat16

B, N, C, H, D = 2, 2048, 1024, 16, 64
NCORES = 8
HL = H // NCORES          # heads per core (2)
BN = B * N                # 4096
NSH = BN // NCORES        # 512 output columns per core
TSH = NSH // B            # 256 tokens per (core, batch)
CCH = C // 128            # 8 contraction chunks
NBH = B * HL              # 4 (batch, local-head) blocks per core
MC = N // 128             # 16 m-chunks per sequence
KC = 67                   # contraction rows used (64 dims + 3 aug)
AX = mybir.AxisListType
ALU = mybir.AluOpType
ACT = mybir.ActivationFunctionType

_compiled = None


def _build():
    nc = bacc.Bacc("TRN2", target_bir_lowering=False, debug=False,
                   num_devices=NCORES)

    x_in = nc.dram_tensor("x_in", [128, CCH, BN], F32R, kind="ExternalInput")
    wq = nc.dram_tensor("wq", [128, CCH, 128], F32R, kind="ExternalInput")
    wk = nc.dram_tensor("wk", [128, CCH, 128], F32R, kind="ExternalInput")
    wv = nc.dram_tensor("wv", [128, CCH, 128], F32R, kind="ExternalInput")
    wp = nc.dram_tensor("wp", [128, CCH, C], BF16, kind="ExternalInput")
    bp_t = nc.dram_tensor("bp_t", [128, CCH], F32, kind="ExternalInput")
    qaug = nc.dram_tensor("qaug", [HL, 3, N], F32R, kind="ExternalInput")
    kaug = nc.dram_tensor("kaug", [HL, 3, N], F32R, kind="ExternalInput")
    mbias = nc.dram_tensor("mbias", [128, HL * MC], F32, kind="ExternalInput")
    ident = nc.dram_tensor("ident", [128, 128], F32, kind="ExternalInput")
    identb = nc.dram_tensor("identb", [128, 128], BF16, kind="ExternalInput")
    out_t = nc.dram_tensor("out", [C, NSH], F32, kind="ExternalOutput")

    with tile.TileContext(nc) as tc:
        with tc.tile_pool(name="wpool", bufs=1) as wpool, \
             tc.tile_pool(name="xpool", bufs=1) as xpool, \
             tc.tile_pool(name="qkpool", bufs=1) as qkpool, \
             tc.tile_pool(name="aux", bufs=2) as aux, \
             tc.tile_pool(name="attp", bufs=1) as attp, \
             tc.tile_pool(name="psum", bufs=1, space="PSUM") as psum, \
             tc.tile_pool(name="dram", bufs=1, space="DRAM") as dram:

            # ---------- resident weights / aux ----------
            wq_sb = wpool.tile([128, CCH, 128], F32R)
            wk_sb = wpool.tile([128, CCH, 128], F32R)
            wv_sb = wpool.tile([128, CCH, 128], F32R)
            wp_sb = wpool.tile([128, CCH, C], BF16)
            bp_sb = wpool.tile([128, CCH], F32)
            mbias_sb = wpool.tile([128, HL * MC], F32)
            ident_sb = wpool.tile([128, 128], F32)
            identb_sb = wpool.tile([128, 128], BF16)
            for sb_t, dr_t in ((wq_sb, wq), (wk_sb, wk), (wv_sb, wv)):
                nc.sync.dma_start(sb_t[:], dr_t[:, :])
            # not needed until attention / output projection: off the hot path
            for sb_t, dr_t in ((ident_sb, ident), (identb_sb, identb),
                               (mbias_sb, mbias), (bp_sb, bp_t), (wp_sb, wp)):
                nc.gpsimd.dma_start(sb_t[:], dr_t[:, :])

            # warmup AllToAll: absorbs the collective cold-start during proj
            wu_sb = aux.tile([8, 16], BF16, tag="wu", name="wu_sb")
            nc.vector.memset(wu_sb[:], 0.0)
            wu_in = dram.tile([NCORES, 16], BF16, name="wu_in")
            wu_out = dram.tile([NCORES, 16], BF16, name="wu_out")
            nc.gpsimd.dma_start(wu_in[:, :], wu_sb[:])
            nc.gpsimd.collective_compute(
                "AllToAll", ALU.bypass,
                replica_groups=[list(range(NCORES))],
                ins=[wu_in.opt()], outs=[wu_out.opt()])

            # ---------- per-(batch, local-head) persistent tiles ----------
            QT, KT, VA, MP = [], [], [], []
            for i in range(NBH):
                j = i % HL
                # rows 0-63 head dims; 64-66 aug rows; 67-127 unused
                q = qkpool.tile([128, N], F32R, name=f"Qt{i}", tag=f"Qt{i}")
                k = qkpool.tile([128, N], F32R, name=f"Kt{i}", tag=f"Kt{i}")
                va = qkpool.tile([128, MC, 65], BF16, name=f"Va{i}", tag=f"Va{i}")
                mp = qkpool.tile([128, 32], F32, name=f"Mp{i}", tag=f"Mp{i}")
                # q rows 64-66: [-slope*n; -M placeholder (0); ones]
                nc.sync.dma_start(q[64:67, :], qaug[j, :, :])
                # k rows 64-66: [ones; ones; f32(fp16(slope*m))]
                nc.sync.dma_start(k[64:67, :], kaug[j, :, :])
                nc.vector.memset(va[:, :, 64:65], 1.0)
                QT.append(q); KT.append(k); VA.append(va); MP.append(mp)

            # ---------- projections (4 pairs of 1024 tokens over B*N) ----------
            def proj_pair(bp_i):
                b = bp_i // 2
                nw = bp_i % 2         # 1024-block within batch
                col0 = bp_i * 1024
                xh = []
                for c in range(CCH):
                    th = xpool.tile([128, 1024], F32R, name=f"x{bp_i}_{c}",
                                    tag="xt", bufs=12)
                    (nc.sync if c % 2 == 0 else nc.scalar).dma_start(
                        th[:], x_in[:, c, col0:col0 + 1024])
                    xh.append(th)

                cols = slice(nw * 1024, nw * 1024 + 1024)
                for ti, (w_t, T) in enumerate(((wq_sb, QT), (wk_sb, KT))):
                    ps = psum.tile([128, 1024], F32, tag="sc", bufs=3,
                                   name=f"pj{bp_i}_{ti}")
                    for c in range(CCH):
                        for half in range(2):
                            hs = slice(half * 512, half * 512 + 512)
                            nc.tensor.matmul(ps[:, hs], w_t[:, c, :],
                                             xh[c][:, hs],
                                             start=(c == 0), stop=(c == CCH - 1))
                        if c == 4:
                            yield
                    for j in range(HL):
                        i = b * HL + j
                        nc.any.tensor_copy(T[i][0:64, cols],
                                           ps[64 * j:64 * j + 64, :])
                    yield

                # v projected in [e,m] layout, then PE-transposed to [m,e]
                vps = psum.tile([128, 1024], F32, tag="sc", bufs=3,
                                name=f"vem{bp_i}")
                for c in range(CCH):
                    for half in range(2):
                        hs = slice(half * 512, half * 512 + 512)
                        nc.tensor.matmul(vps[:, hs], wv_sb[:, c, :],
                                         xh[c][:, hs],
                                         start=(c == 0), stop=(c == CCH - 1))
                    if c == 4:
                        yield
                vtmp = xpool.tile([128, 1024], BF16, tag="vt", bufs=2,
                                  name=f"vtmp{bp_i}")
                nc.any.tensor_copy(vtmp[:], vps[:])
                yield
                for mt in range(8):
                    vtp = psum.tile([128, 128], BF16, tag="sc", bufs=3,
                                    name=f"vtp{bp_i}_{mt}")
                    nc.tensor.transpose(vtp[:], vtmp[:, mt * 128:(mt + 1) * 128],
                                        identb_sb[:])
                    mc = nw * 8 + mt
                    for j in range(HL):
                        i = b * HL + j
                        nc.any.tensor_copy(VA[i][:, mc, 0:64],
                                           vtp[:, 64 * j:64 * j + 64])
                    if mt % 2 == 1:
                        yield

            # ---------- attention ----------
            # per-block head-split AllToAll buffers: block i = (b, j) sends
            # head (2*core+j), batch-b rows to all destination cores
            ag_in = [dram.tile([NCORES, 64, TSH], BF16, name=f"agi{i}")
                     for i in range(NBH)]
            ag_out = [dram.tile([NCORES, 64, TSH], BF16, name=f"ago{i}")
                      for i in range(NBH)]
            # gathered attention outputs for my tokens: [channel, token]
            # cols 0:256 batch-0 tokens, 256:512 batch-1 tokens
            gt_in = attp.tile([128, CCH, NSH], BF16, tag="gtin", bufs=1)

            def pass1(i):
                Q, K, Mpt = QT[i], KT[i], MP[i]
                for nt in range(16):
                    for half in range(2):
                        ps = psum.tile([128, 1024], F32, tag="sc", bufs=3,
                                       name=f"p1_{i}_{nt}_{half}")
                        for mb in range(2):
                            m0 = half * 1024 + mb * 512
                            nc.tensor.matmul(ps[:, mb * 512:(mb + 1) * 512],
                                             Q[0:KC, nt * 128:(nt + 1) * 128],
                                             K[0:KC, m0:m0 + 512],
                                             start=True, stop=True)
                        nc.vector.tensor_reduce(
                            Mpt[:, nt * 2 + half:nt * 2 + half + 1], ps[:, :],
                            axis=AX.X, op=ALU.max)
                        yield
                mneg = aux.tile([128, 16], F32, tag="mneg", name=f"mneg{i}")
                nc.vector.tensor_reduce(
                    mneg[:], Mpt[:].rearrange("p (a b) -> p a b", b=2),
                    axis=AX.X, op=ALU.max, negate=True)
                trp = psum.tile([16, 128], F32, tag="sc", bufs=3, name=f"trp{i}")
                nc.tensor.transpose(trp[:], mneg[:], ident_sb[:])
                mrow = aux.tile([16, 128], F32R, tag="mrow", name=f"mr{i}")
                nc.any.tensor_copy(mrow[:], trp[:])
                nc.gpsimd.dma_start(QT[i][65:66, :], mrow[:, :])
                yield

            def pass2(i):
                b, j = divmod(i, HL)
                Q, K, Va = QT[i], KT[i], VA[i]
                for nb in range(2):
                    n0 = nb * 1024
                    avp = [psum.tile([65, 512], F32, tag="avp", bufs=2,
                                     name=f"av_{i}_{nb}_{hf}")
                           for hf in range(2)]
                    at_q = []

                    def emit_av(mc, at):
                        for hf in range(2):
                            hs = slice(hf * 512, hf * 512 + 512)
                            nc.tensor.matmul(avp[hf][:], Va[:, mc, :], at[:, hs],
                                             start=(mc == 0), stop=(mc == MC - 1))

                    for mc in range(MC):
                        s2 = psum.tile([128, 1024], F32, tag="sc", bufs=3,
                                       name=f"s2_{i}_{nb}_{mc}")
                        for hf in range(2):
                            hs = slice(hf * 512, hf * 512 + 512)
                            nc.tensor.matmul(s2[:, hs],
                                             K[0:KC, mc * 128:(mc + 1) * 128],
                                             Q[0:KC, n0 + hf * 512:n0 + hf * 512 + 512],
                                             start=True, stop=True)
                        at = attp.tile([128, 1024], BF16, tag="att", bufs=4,
                                       name=f"at_{i}_{nb}_{mc}")
                        nc.scalar.activation(
                            at[:], s2[:], ACT.Exp,
                            bias=mbias_sb[:, j * MC + mc:j * MC + mc + 1],
                            scale=1.0)
                        # av for the PREVIOUS chunk: its exp has had a full
                        # s2 round to drain, so the PE never waits on ACT
                        at_q.append((mc, at))
                        if len(at_q) > 1:
                            emit_av(*at_q.pop(0))
                        yield
                    emit_av(*at_q.pop(0))
                    # normalize per half: reciprocal spread over 32 partitions
                    for hf in range(2):
                        sfx = f"{i}_{nb}_{hf}"
                        lrow = aux.tile([1, 512], F32, tag="lrow", bufs=2,
                                        name=f"lr_{sfx}")
                        nc.any.tensor_copy(lrow[0:1, :], avp[hf][64:65, :])
                        l32 = aux.tile([32, 16], F32, tag="l32", bufs=2,
                                       name=f"l32_{sfx}")
                        nc.gpsimd.dma_start(l32[:, :], lrow[0:1, :])
                        r32 = aux.tile([32, 16], F32, tag="r32", bufs=2,
                                       name=f"r32_{sfx}")
                        nc.vector.reciprocal(r32[:], l32[:])
                        linv = aux.tile([1, 512], F32, tag="linv", bufs=2,
                                        name=f"li_{sfx}")
                        nc.gpsimd.dma_start(linv[0:1, :], r32[:, :])
                        lb = aux.tile([64, 512], F32, tag="lb", bufs=2,
                                      name=f"lb_{sfx}")
                        nc.gpsimd.partition_broadcast(lb[:], linv[0:1, :])
                        gt = aux.tile([64, 512], BF16, tag="gt", bufs=2,
                                      name=f"gt_{sfx}")
                        nc.vector.tensor_mul(gt[:], avp[hf][0:64, :], lb[:])
                        for sf in range(2):
                            s = nb * 4 + hf * 2 + sf
                            (nc.sync if sf == 0 else nc.scalar).dma_start(
                                ag_in[i][s, :, :], gt[:, sf * 256:sf * 256 + 256])
                        yield

            def emit_a2a(i):
                nc.gpsimd.collective_compute(
                    "AllToAll", ALU.bypass,
                    replica_groups=[list(range(NCORES))],
                    ins=[ag_in[i].opt()],
                    outs=[ag_out[i].opt()],
                )

            def redistribute(i):
                b, j = divmod(i, HL)
                rows = slice(64 * j, 64 * j + 64)
                csl = slice(TSH * b, TSH * b + TSH)
                for c in range(CCH):
                    (nc.sync if c % 2 == 0 else nc.scalar).dma_start(
                        gt_in[rows, c, csl], ag_out[i][c, :, :])

            def outproj(half):
                csl = slice(half * TSH, half * TSH + TSH)
                for et in range(CCH):
                    yps = psum.tile([128, TSH], F32, tag="sc", bufs=3,
                                    name=f"y{half}_{et}")
                    for c in range(CCH):
                        nc.tensor.matmul(yps[:],
                                         wp_sb[:, c, et * 128:(et + 1) * 128],
                                         gt_in[:, c, csl],
                                         start=(c == 0), stop=(c == CCH - 1))
                    ysb = aux.tile([128, TSH], F32, tag="y", name=f"ysb{half}_{et}")
                    nc.scalar.activation(ysb[:], yps[:], ACT.Identity,
                                         bias=bp_sb[:, et:et + 1], scale=1.0)
                    nc.sync.dma_start(out_t[et * 128:(et + 1) * 128, csl], ysb[:])
                    yield

            def drive(*gens_weights):
                """Round-robin generators with weights until all exhausted."""
                gens = [[g, w] for g, w in gens_weights]
                while gens:
                    for gw in list(gens):
                        g, w = gw
                        for _ in range(w):
                            try:
                                next(g)
                            except StopIteration:
                                gens.remove(gw)
                                break

            def chain(*gens):
                for g in gens:
                    yield from g

            # batch-0 proj; then batch-1 proj interleaved with batch-0 pass1;
            # then pass2 blocks in order, each firing its own AllToAll;
            # outproj half 0 rides the last collective's flight window
            drive((chain(proj_pair(0), proj_pair(1)), 1))
            drive((chain(proj_pair(2), proj_pair(3)), 1),
                  (chain(pass1(0), pass1(1)), 2))
            drive((pass2(0), 1), (chain(pass1(2), pass1(3)), 1))
            emit_a2a(0)
            drive((pass2(1), 1))
            emit_a2a(1)
            drive((pass2(2), 1))
            emit_a2a(2)
            drive((pass2(3), 1))
            emit_a2a(3)
            redistribute(0)
            redistribute(1)
            drive((outproj(0), 1))
            redistribute(2)
            redistribute(3)
            drive((outproj(1), 1))

    nc.compile()
    return nc


def _get_nc():
    global _compiled
    if _compiled is None:
        _compiled = _build()
    return _compiled


def _alibi_slopes():
    x = (2 ** 8) ** (1.0 / H)
    return np.array([1.0 / x ** (i + 1) for i in range(H)], dtype=np.float64)


def _chunked(a):
    """[C, F] -> [128, CCH, F] (partition, c-chunk, free)."""
    Cdim, F = a.shape
    return np.ascontiguousarray(a.reshape(CCH, 128, F).transpose(1, 0, 2))


def _make_in_maps(x, Wq, Wk, Wv, Wp, bp):
    x = np.asarray(x, dtype=np.float32)
    xT = np.ascontiguousarray(x.reshape(BN, C).T)          # [C, BN]
    xch = _chunked(xT)

    slopes = _alibi_slopes()
    n_arr = np.arange(N, dtype=np.float64)
    p_arr = np.arange(128, dtype=np.float64)

    wp_ch = _chunked(np.ascontiguousarray(np.asarray(Wp, np.float32).T)).astype(BF)
    bp_tile = np.ascontiguousarray(
        np.asarray(bp, np.float32).reshape(CCH, 128).T)
    identity = np.eye(128, dtype=np.float32)

    in_maps = []
    for core in range(NCORES):
        e0 = core * 128
        wqT = np.ascontiguousarray((8.0 * np.asarray(Wq, np.float32)[e0:e0 + 128]).T)
        wkT = np.ascontiguousarray(np.asarray(Wk, np.float32)[e0:e0 + 128].T)
        wvT = np.ascontiguousarray(np.asarray(Wv, np.float32)[e0:e0 + 128].T)

        s = slopes[core * HL: core * HL + HL]               # [HL]
        qa = np.zeros((HL, 3, N), dtype=np.float32)
        ka = np.zeros((HL, 3, N), dtype=np.float32)
        # pass2's matmul adds fp16(slope*m) via k row 66 (fp16 values are
        # exact in fp32r); the exp bias supplies the fp32 residual
        mb = np.zeros((128, HL * MC), dtype=np.float32)
        for j in range(HL):
            qa[j, 0] = (-s[j] * n_arr).astype(np.float32)   # -slope*n
            qa[j, 1] = 0.0                                  # -M placeholder
            qa[j, 2] = 1.0
            ka[j, 0] = 1.0
            ka[j, 1] = 1.0
            ka[j, 2] = (s[j] * n_arr).astype(np.float16).astype(np.float32)
            for c in range(MC):
                exact = (s[j] * (128 * c + p_arr))
                mb[:, j * MC + c] = (exact -
                                     exact.astype(np.float16).astype(np.float64)
                                     ).astype(np.float32)

        in_maps.append({
            "x_in": xch,
            "wq": _chunked(wqT), "wk": _chunked(wkT), "wv": _chunked(wvT),
            "wp": wp_ch, "bp_t": bp_tile,
            "qaug": qa, "kaug": ka, "mbias": mb,
            "ident": identity, "identb": identity.astype(BF),
        })
    return in_maps


def run(x, Wq, Wk, Wv, Wp, bp, trace=False, tmpdir=None):
    nc = _get_nc()
    in_maps = _make_in_maps(x, Wq, Wk, Wv, Wp, bp)
    kwargs = {}
    if trace:
        kwargs = {"trace": True, "tmpdir": tmpdir}
    res = run_bass_kernel_spmd(nc, in_maps, core_ids=list(range(NCORES)), **kwargs)
    # core i columns: [0:256] = batch-0 tokens [256i,256i+256),
    #                 [256:512] = batch-1 tokens [256i,256i+256)
    out = np.empty((B, N, C), dtype=np.float32)
    for i in range(NCORES):
        yT = res.results[i]["out"]                          # [C, NSH]
        for b in range(B):
            out[b, TSH * i:TSH * i + TSH, :] = yT[:, b * TSH:(b + 1) * TSH].T
    return out, res


def kernel(x, Wq, Wk, Wv, Wp, bp):
    out, _ = run(x, Wq, Wk, Wv, Wp, bp)
    return out
